# revision 1
# baseline (speedup 1.0000x reference)
import sys
import os

sys.path.insert(0, "/opt/trn_rl_repo")

import numpy as np
import ml_dtypes

import concourse.bass as bass
import concourse.tile as tile
from concourse import mybir, library_config
from concourse.tile import add_dep_helper
from concourse.bass_utils import run_bass_kernel_spmd

# Problem constants (nn_MoEBlock: B,C,T,H,W = 2,128,8,64,64; E=8; top-2)
B, C, T, H, W = 2, 128, 8, 64, 64
E = 8
NVOX = B * T * H * W          # 65536 voxels
NCORES = 8
NSH = NVOX // NCORES          # 8192 voxels per core
NC_CHUNK = 1024               # main-loop chunk (voxels)
NCHUNKS = NSH // NC_CHUNK
F32 = mybir.dt.float32
F32R = mybir.dt.float32r
NEG_BIG = -1e30


def _r(ap):
    return ap.bitcast(F32R)



def _split_waits(nc, max_waits=1):
    """This walrus accepts only one sync-wait command per instruction.
    Move extra on_wait conditions onto standalone same-engine NoOps
    inserted immediately before the instruction (same engine stream =>
    identical semantics)."""
    ctr = 0
    for f in nc.m.functions:
        for bb in f.blocks:
            insts = list(bb.instructions)
            out = []
            changed = False
            for inst in insts:
                si = inst.sync_info
                w = list(si.on_wait) if si is not None and si.on_wait else []
                if (len(w) > max_waits
                        and inst.engine != mybir.EngineType.Unassigned):
                    for extra in w[:-max_waits]:
                        ctr += 1
                        nop = mybir.InstNoOp(
                            name=f"WSPLIT-{ctr}", ins=[], outs=[])
                        nop.engine = inst.engine
                        nop.sync_info = mybir.SyncInfo(
                            on_wait=[extra], on_update=[])
                        out.append(nop)
                    inst.sync_info = mybir.SyncInfo(
                        on_wait=w[-max_waits:],
                        on_update=list(si.on_update) if si.on_update else [])
                    changed = True
                out.append(inst)
            if changed:
                try:
                    bb.instructions = out
                except Exception:
                    bb.instructions.clear()
                    bb.instructions.extend(out)
    return nc


def build_kernel(hasgb: bool, hasb1: bool, hasb2: bool, act_fn=None):
    if act_fn is None:
        act_fn = mybir.ActivationFunctionType.Silu
    nc = bass.Bass()
    x_d = nc.dram_tensor("x", [C, NSH], F32R, kind="ExternalInput")
    gwT_d = nc.dram_tensor("gwT", [C, E], F32R, kind="ExternalInput")
    gb_d = nc.dram_tensor("gb", [C, E], F32, kind="ExternalInput")
    w1T_d = nc.dram_tensor("w1T", [C, E * C], F32R, kind="ExternalInput")
    b1_d = nc.dram_tensor("b1m", [C, E], F32, kind="ExternalInput")
    w2T_d = nc.dram_tensor("w2T", [C, E * C], mybir.dt.bfloat16, kind="ExternalInput")
    b2_d = nc.dram_tensor("b2m", [E, C], mybir.dt.bfloat16, kind="ExternalInput")
    id_d = nc.dram_tensor("ident", [C, C], F32, kind="ExternalInput")
    sel_d = nc.dram_tensor("sel", [E, E * C], mybir.dt.bfloat16, kind="ExternalInput")
    out_d = nc.dram_tensor("out", [C, NSH], F32, kind="ExternalOutput")

    with tile.TileContext(nc) as tc:
        with (
            tc.tile_pool(name="consts", bufs=1) as consts,
            tc.tile_pool(name="xp", bufs=1) as xp,
            tc.tile_pool(name="gat", bufs=1) as gat,
            tc.tile_pool(name="fpool", bufs=3) as fpool,
            tc.tile_pool(name="gpool", bufs=3) as gpool,
            tc.tile_pool(name="opool", bufs=2) as opool,
        ):
            # ---------- phase 0: loads ----------
            x_sb = xp.tile([C, NSH], F32R)
            gwT = consts.tile([C, E], F32R)
            gbr = consts.tile([C, E], F32)
            w1T = consts.tile([C, E * C], F32R)
            b1m = consts.tile([C, E], F32)
            w2T = consts.tile([C, E * C], mybir.dt.bfloat16)
            b2m = consts.tile([E, C], mybir.dt.bfloat16)
            ident = consts.tile([C, C], F32)
            scal1 = consts.tile([C, 1], F32)
            sel = consts.tile([E, E * C], mybir.dt.bfloat16)

            dmas = []
            for j in range(4):
                s = slice(j * (NSH // 4), (j + 1) * (NSH // 4))
                dmas.append(nc.sync.dma_start(x_sb[:, s], x_d[:, s]))
            dmas.append(nc.sync.dma_start(gwT[:], gwT_d[:]))
            dmas.append(nc.sync.dma_start(gbr[:], gb_d[:]))
            dmas.append(nc.sync.dma_start(w1T[:], w1T_d[:]))
            dmas.append(nc.sync.dma_start(b1m[:], b1_d[:]))
            dmas.append(nc.sync.dma_start(w2T[:], w2T_d[:]))
            dmas.append(nc.sync.dma_start(b2m[:], b2_d[:]))
            dmas.append(nc.sync.dma_start(ident[:], id_d[:]))
            nc.vector.memset(scal1[:], 1.0)
            dmas.append(nc.sync.dma_start(sel[:], sel_d[:]))

            # PE can carry only ONE sync wait per Matmult through walrus;
            # absorb each input-DMA dependency into a PE nop up front.
            dma_nops = []
            for dma in dmas:
                nop = nc.tensor.nop(nofuse=True)
                add_dep_helper(nop.ins, dma.ins, sync=True)
                dma_nops.append(nop)

            def pe_absorb(producers, consumer_mms):
                nops = []
                for p in producers:
                    if p is None:
                        continue
                    n = nc.tensor.nop(nofuse=True)
                    add_dep_helper(n.ins, p.ins, sync=True)
                    nops.append(n)
                for m in consumer_mms:
                    for n in nops:
                        add_dep_helper(m.ins, n.ins, sync=False)

            # ---------- phase G: gating ----------
            gpsum = tc.tile_pool(name="ps_g", bufs=1, space="PSUM")
            ps_l = gpsum.__enter__()
            NT = NSH // C  # 64 voxel tiles of 128
            psl = ps_l.tile([C, NT * E], F32)   # [128, 512] logits, voxel-major
            for i in range(NT):
                lmm = nc.tensor.matmul(
                    psl[:, i * E:(i + 1) * E],
                    x_sb[:, i * C:(i + 1) * C].bitcast(F32),
                    gwT[:].bitcast(F32),
                    start=True, stop=True,
                )
                if i == 0:
                    for n in dma_nops:
                        add_dep_helper(lmm.ins, n.ins, sync=False)
            l3 = psl[:].rearrange("p (t e) -> p t e", e=E)
            if hasgb:
                lsb = gat.tile([C, NT * E], F32)
                nc.vector.tensor_add(
                    lsb[:].rearrange("p (t e) -> p t e", e=E), l3,
                    gbr[:, None, :].broadcast_to((C, NT, E)))
                l3 = lsb[:].rearrange("p (t e) -> p t e", e=E)

            m1 = gat.tile([C, NT], F32)
            nc.vector.tensor_reduce(
                out=m1[:], in_=l3, op=mybir.AluOpType.max, axis=mybir.AxisListType.X)
            ge1 = gat.tile([C, NT * E], F32)
            g13 = ge1[:].rearrange("p (t e) -> p t e", e=E)
            nc.vector.tensor_tensor(
                g13, l3, m1[:, :, None].broadcast_to((C, NT, E)),
                op=mybir.AluOpType.is_ge)
            tneg = gat.tile([C, NT * E], F32)
            nc.vector.tensor_scalar_mul(tneg[:], ge1[:], NEG_BIG)
            lm = gat.tile([C, NT * E], F32)
            nc.vector.tensor_add(
                lm[:].rearrange("p (t e) -> p t e", e=E), l3,
                tneg[:].rearrange("p (t e) -> p t e", e=E))
            lm3 = lm[:].rearrange("p (t e) -> p t e", e=E)
            m2 = gat.tile([C, NT], F32)
            nc.vector.tensor_reduce(
                out=m2[:], in_=lm3, op=mybir.AluOpType.max, axis=mybir.AxisListType.X)
            ge2 = gat.tile([C, NT * E], F32)
            nc.vector.tensor_tensor(
                ge2[:].rearrange("p (t e) -> p t e", e=E), lm3,
                m2[:, :, None].broadcast_to((C, NT, E)),
                op=mybir.AluOpType.is_ge)
            dd = gat.tile([C, NT], F32)
            nc.vector.tensor_sub(dd[:], m2[:], m1[:])
            # sigmoid is not in the HW act-table set that has silu; use
            # sigmoid(z) = 0.5 + 0.5*tanh(z/2) (tanh shares silu's table).
            th = gat.tile([C, NT], F32)
            nc.scalar.activation(
                th[:], dd[:], mybir.ActivationFunctionType.Tanh, scale=-0.5)
            p1 = gat.tile([C, NT], F32)  # sigmoid(m1-m2)
            nc.vector.tensor_scalar(
                out=p1[:], in0=th[:], scalar1=0.5, scalar2=0.5,
                op0=mybir.AluOpType.mult, op1=mybir.AluOpType.add)
            p2 = gat.tile([C, NT], F32)  # sigmoid(m2-m1)
            nc.vector.tensor_scalar(
                out=p2[:], in0=th[:], scalar1=-0.5, scalar2=0.5,
                op0=mybir.AluOpType.mult, op1=mybir.AluOpType.add)
            # wt = ge1*p1 + ge2*p2  (voxel-major top-2 softmax weights)
            nc.vector.tensor_mul(
                g13, g13, p1[:, :, None].broadcast_to((C, NT, E)))
            nc.vector.tensor_mul(
                ge2[:].rearrange("p (t e) -> p t e", e=E),
                ge2[:].rearrange("p (t e) -> p t e", e=E),
                p2[:, :, None].broadcast_to((C, NT, E)))
            wt = gat.tile([C, NT * E], F32)
            wt_ins = nc.vector.tensor_add(wt[:], ge1[:], ge2[:])

            # transpose wt [128,(t,8)] -> channel-major [8, NSH] via PE,
            # bounce through SBUF (PSUM cannot be DMA'd) to DRAM
            wcm_sb = consts.tile([E, NSH], mybir.dt.bfloat16)
            TJ = 16  # psum transposes batched 4 per bank
            prev_copy = None
            prev_grp = []
            for j in range(TJ):
                pst = ps_l.tile([E, 4 * C], F32, tag="pst")
                grp = []
                for k in range(4):
                    ti = 4 * j + k
                    grp.append(nc.tensor.transpose(
                        pst[:, k * C:(k + 1) * C],
                        wt[:, ti * E:(ti + 1) * E],
                        ident[:]))
                pe_absorb([wt_ins if j == 0 else None, prev_copy,
                           prev_grp[-1] if prev_grp else None], grp[:1])
                for m in grp[1:]:
                    add_dep_helper(m.ins, grp[0].ins, sync=False)
                prev_copy = nc.scalar.copy(
                    wcm_sb[:, j * 4 * C:(j + 1) * 4 * C], pst[:])
                prev_grp = grp
            gpsum.__exit__(None, None, None)

            # ---------- phase M: experts + combine ----------
            mpsum = tc.tile_pool(name="ps_m", bufs=1, space="PSUM")
            ps_m = mpsum.__enter__()
            mpsum2 = tc.tile_pool(name="ps_m2", bufs=2, space="PSUM")
            ps_m2 = mpsum2.__enter__()
            prev_resid = prev_l2last = None
            hist_silu = [None, None]
            hist_mult = [None, None]
            hist_hmm = [None, None]
            hist_wb = [None, None]
            for i in range(NCHUNKS):
                cs = slice(i * NC_CHUNK, (i + 1) * NC_CHUNK)
                pso = ps_m.tile([C, NC_CHUNK], F32, tag="pso")
                for e in range(E):
                    mms = []
                    psh = ps_m2.tile([C, NC_CHUNK], F32, tag="psh")
                    for s in range(NC_CHUNK // 512):
                        rs = slice(i * NC_CHUNK + s * 512,
                                   i * NC_CHUNK + (s + 1) * 512)
                        mms.append(nc.tensor.matmul(
                            psh[:, s * 512:(s + 1) * 512],
                            w1T[:, e * C:(e + 1) * C],
                            x_sb[:, rs],
                            start=True, stop=True))
                    f_sb = fpool.tile([C, NC_CHUNK], F32, tag="f")
                    if hasb1:
                        silu_ins = nc.scalar.activation(
                            f_sb[:], psh[:], act_fn, bias=b1m[:, e:e + 1])
                    else:
                        silu_ins = nc.scalar.activation(f_sb[:], psh[:], act_fn)
                    pswb = ps_m.tile([C, NC_CHUNK], F32, tag="pswb")
                    for s in range(NC_CHUNK // 512):
                        ws = slice(i * NC_CHUNK + s * 512,
                                   i * NC_CHUNK + (s + 1) * 512)
                        mms.append(nc.tensor.matmul(
                            pswb[:, s * 512:(s + 1) * 512],
                            sel[:, e * C:(e + 1) * C], wcm_sb[:, ws],
                            start=True, stop=True))
                    g_sb = gpool.tile([C, NC_CHUNK], mybir.dt.bfloat16, tag="g")
                    mult_ins = nc.vector.tensor_mul(g_sb[:], f_sb[:], pswb[:])
                    for s in range(NC_CHUNK // 512):
                        ss = slice(s * 512, (s + 1) * 512)
                        mms.append(nc.tensor.matmul(
                            pso[:, ss],
                            w2T[:, e * C:(e + 1) * C],
                            g_sb[:, ss],
                            start=(e == 0),
                            stop=(e == E - 1) and not hasb2))
                    # absorb all cross-engine + psum-WAW deps into PE nops
                    pe_absorb([hist_silu[0], hist_mult[-1], prev_resid,
                               hist_hmm[0], hist_wb[-1], prev_l2last],
                              mms[:1])
                    for m in mms[1:]:
                        add_dep_helper(m.ins, mms[0].ins, sync=False)
                    pe_absorb([silu_ins, mult_ins], mms[-NC_CHUNK // 512:])
                    hist_silu = [hist_silu[-1], silu_ins]
                    hist_mult = [hist_mult[-1], mult_ins]
                    hist_hmm = [hist_hmm[-1], mms[1]]
                    hist_wb = [hist_wb[-1], mms[NC_CHUNK // 512 + 1]]
                    if e == E - 1:
                        prev_l2last = mms[-1]
                if hasb2:
                    for s in range(NC_CHUNK // 512):
                        ss = slice(s * 512, (s + 1) * 512)
                        rs = slice(i * NC_CHUNK + s * 512,
                                   i * NC_CHUNK + (s + 1) * 512)
                        nc.tensor.matmul(
                            pso[:, ss], b2m[:], wcm_sb[:, rs],
                            start=False, stop=True)
                o_sb = opool.tile([C, NC_CHUNK], F32, tag="o")
                prev_resid = nc.vector.tensor_add(
                    o_sb[:], pso[:], x_sb[:, cs].bitcast(F32))
                nc.sync.dma_start(out_d[:, cs], o_sb[:])
            mpsum2.__exit__(None, None, None)
            mpsum.__exit__(None, None, None)
    _split_waits(nc)
    return nc


_cache = {}


def _get_nc(key):
    if key not in _cache:
        _cache[key] = build_kernel(*key)
    return _cache[key]


def kernel(x, gate_w, gate_b, w1, b1, w2, b2, _trace=False):
    x = np.asarray(x, dtype=np.float32)
    gate_w = np.asarray(gate_w, dtype=np.float32)
    gate_b = np.asarray(gate_b, dtype=np.float32)
    w1 = np.asarray(w1, dtype=np.float32)
    b1 = np.asarray(b1, dtype=np.float32)
    w2 = np.asarray(w2, dtype=np.float32)
    b2 = np.asarray(b2, dtype=np.float32)

    x_cm = np.ascontiguousarray(
        x.transpose(1, 0, 2, 3, 4).reshape(C, NVOX))
    gwT = np.ascontiguousarray(gate_w.T)                      # [C, E]
    gbr = np.tile(gate_b[None, :], (C, 1)).astype(np.float32)  # [C, E]
    w1T = np.ascontiguousarray(w1.T)                          # [C, E*C]
    b1m = np.ascontiguousarray(b1.reshape(E, C).T)            # [C, E]
    w2T = np.ascontiguousarray(
        w2.transpose(2, 0, 1).reshape(C, E * C)).astype(ml_dtypes.bfloat16)
    b2m = np.ascontiguousarray(b2).astype(ml_dtypes.bfloat16)
    ident = np.eye(C, dtype=np.float32)
    sel = np.zeros((E, E * C), dtype=ml_dtypes.bfloat16)
    for e in range(E):
        sel[e, e * C:(e + 1) * C] = 1.0

    key = (bool(gate_b.any()), bool(b1.any()), bool(b2.any()))
    nc = _get_nc(key)

    in_maps = []
    for c in range(NCORES):
        sh = np.ascontiguousarray(x_cm[:, c * NSH:(c + 1) * NSH])
        in_maps.append({
            "x": sh, "gwT": gwT, "gb": gbr, "w1T": w1T, "b1m": b1m,
            "w2T": w2T, "b2m": b2m, "ident": ident, "sel": sel,
        })

    res = run_bass_kernel_spmd(
        nc, in_maps, core_ids=list(range(NCORES)), trace=_trace)
    out_cm = np.concatenate([res.results[c]["out"] for c in range(NCORES)],
                            axis=1)
    out = out_cm.reshape(C, B, T, H, W).transpose(1, 0, 2, 3, 4)
    out = np.ascontiguousarray(out, dtype=np.float32)
    if _trace:
        return out, res
    return out



# revision 17
# speedup vs baseline: 5.9614x; 5.9614x over previous
import sys

sys.path.insert(0, "/opt/trn_rl_repo")

import numpy as np

import jax
import jax.numpy as jnp
from jax.sharding import Mesh, NamedSharding, PartitionSpec
from jax.experimental.shard_map import shard_map

import concourse.bass as bass
import concourse.tile as tile
from concourse import mybir
from concourse.tile import add_dep_helper
from concourse.bass2jax import (
    _bass_exec_p, install_neuronx_cc_hook, partition_id_tensor)

# Problem constants (nn_MoEBlock: B,C,T,H,W = 2,128,8,64,64; E=8; top-2)
B, C, T, H, W = 2, 128, 8, 64, 64
E = 8
THW = T * H * W               # 32768
NVOX = B * THW                # 65536 voxels
NCORES = 8
NSH = NVOX // NCORES          # 8192 voxels per core
CPB = NCORES // B             # cores per batch element (4)
NSPLIT = 1                    # calls per kernel() invocation (pipelining
                              # deeper doesn't help: per-device queues
                              # serialize uploads behind pending executes)
NS = NSH // NSPLIT            # voxels per core per call
NC_CHUNK = 1024               # main-loop chunk (voxels)
NCHUNKS = NS // NC_CHUNK
F16 = mybir.dt.float16
F32 = mybir.dt.float32
F8 = mybir.dt.float8e4
NP_F8 = mybir.dt.np(F8)       # ml_dtypes.float8_e4m3


def _split_waits(nc, max_waits=1):
    """The walrus scheduler accepts only one sync-wait per instruction.
    Move extra on_wait conditions onto standalone same-engine NoOps
    inserted immediately before the instruction (same engine stream =>
    identical semantics)."""
    ctr = 0
    for f in nc.m.functions:
        for bb in f.blocks:
            insts = list(bb.instructions)
            out = []
            changed = False
            for inst in insts:
                si = inst.sync_info
                w = list(si.on_wait) if si is not None and si.on_wait else []
                if (len(w) > max_waits
                        and inst.engine != mybir.EngineType.Unassigned):
                    for extra in w[:-max_waits]:
                        ctr += 1
                        nop = mybir.InstNoOp(
                            name=f"WSPLIT-{ctr}", ins=[], outs=[])
                        nop.engine = inst.engine
                        nop.sync_info = mybir.SyncInfo(
                            on_wait=[extra], on_update=[])
                        out.append(nop)
                    inst.sync_info = mybir.SyncInfo(
                        on_wait=w[-max_waits:],
                        on_update=list(si.on_update) if si.on_update else [])
                    changed = True
                out.append(inst)
            if changed:
                try:
                    bb.instructions = out
                except Exception:
                    bb.instructions.clear()
                    bb.instructions.extend(out)
    return nc


def build_kernel(hasb1: bool, hasb2: bool):
    """Expert MLP only — gating (logits/top-2/softmax) happens on host in
    exact f32; the per-voxel expert weights arrive as the f8 `wc` input.
    Output is the MoE delta (no +x residual; host adds it in f32)."""
    act_fn = mybir.ActivationFunctionType.Silu
    wt_cols = 2 * E * C + (E if hasb1 else 0)
    nc = bass.Bass()
    x8_d = nc.dram_tensor("x8", [C, NS], F8, kind="ExternalInput")
    wc_d = nc.dram_tensor("wc", [E, NS], F8, kind="ExternalInput")
    wt_d = nc.dram_tensor("wt", [C, wt_cols], F16, kind="ExternalInput")
    b2_d = (nc.dram_tensor("b2m", [E, C], F16, kind="ExternalInput")
            if hasb2 else None)
    out_d = nc.dram_tensor("out", [C, NS], F8, kind="ExternalOutput")

    with tile.TileContext(nc) as tc:
        with (
            tc.tile_pool(name="consts", bufs=1) as consts,
            tc.tile_pool(name="xp", bufs=1) as xp,
            tc.tile_pool(name="fpool", bufs=3) as fpool,
            tc.tile_pool(name="gpool", bufs=3) as gpool,
            tc.tile_pool(name="opool", bufs=2) as opool,
        ):
            # ---------- loads ----------
            x_sb = xp.tile([C, NS], F8)
            wcm_sb = consts.tile([E, NS], F8)
            w1T = consts.tile([C, E * C], F16)
            w2T = consts.tile([C, E * C], F16)
            sel32 = consts.tile([E, E * C], F32)
            sel = consts.tile([E, E * C], F16)

            dmas = []
            for j in range(4):
                s = slice(j * (NS // 4), (j + 1) * (NS // 4))
                dmas.append(nc.sync.dma_start(x_sb[:, s], x8_d[:, s]))
            dmas.append(nc.sync.dma_start(wcm_sb[:], wc_d[:]))
            dmas.append(nc.sync.dma_start(w1T[:], wt_d[:, 0:E * C]))
            dmas.append(nc.sync.dma_start(w2T[:], wt_d[:, E * C:2 * E * C]))
            if hasb1:
                b1m = consts.tile([C, E], F32)
                bh = consts.tile([C, E], F16)
                dmas.append(nc.sync.dma_start(
                    bh[:], wt_d[:, 2 * E * C:2 * E * C + E]))
                nc.scalar.copy(b1m[:], bh[:])
            if hasb2:
                b2m = consts.tile([E, C], F16)
                dmas.append(nc.sync.dma_start(b2m[:], b2_d[:]))

            # on-device constant: expert-broadcast selector
            # sel[e, e*C:(e+1)*C] = 1, else 0
            nc.vector.memset(sel32[:], 1.0)
            nc.gpsimd.affine_select(
                sel32[:], sel32[:], pattern=[[1, E * C]], base=0,
                channel_multiplier=-C,
                compare_op=mybir.AluOpType.is_ge, fill=0.0)
            nc.gpsimd.affine_select(
                sel32[:], sel32[:], pattern=[[-1, E * C]], base=C - 1,
                channel_multiplier=C,
                compare_op=mybir.AluOpType.is_ge, fill=0.0)
            sel_cp = nc.scalar.copy(sel[:], sel32[:])

            # PE carries only ONE sync wait per Matmult through walrus;
            # absorb each input-DMA dependency into a PE nop up front.
            dma_nops = []
            for dma in dmas:
                nop = nc.tensor.nop(nofuse=True)
                add_dep_helper(nop.ins, dma.ins, sync=True)
                dma_nops.append(nop)

            def pe_absorb(producers, consumer_mms):
                nops = []
                for p in producers:
                    if p is None:
                        continue
                    n = nc.tensor.nop(nofuse=True)
                    add_dep_helper(n.ins, p.ins, sync=True)
                    nops.append(n)
                for m in consumer_mms:
                    for n in nops:
                        add_dep_helper(m.ins, n.ins, sync=False)

            # ---------- experts + combine ----------
            mpsum = tc.tile_pool(name="ps_m", bufs=1, space="PSUM")
            ps_m = mpsum.__enter__()
            mpsum2 = tc.tile_pool(name="ps_m2", bufs=2, space="PSUM")
            ps_m2 = mpsum2.__enter__()
            prev_out = prev_l2last = None
            hist_silu = [None, None]
            hist_mult = [None, None]
            hist_hmm = [None, None]
            hist_wb = [None, None]
            for i in range(NCHUNKS):
                cs = slice(i * NC_CHUNK, (i + 1) * NC_CHUNK)
                pso = ps_m.tile([C, NC_CHUNK], F32, tag="pso")
                for e in range(E):
                    mms = []
                    psh = ps_m2.tile([C, NC_CHUNK], F32, tag="psh")
                    for s in range(NC_CHUNK // 512):
                        rs = slice(i * NC_CHUNK + s * 512,
                                   i * NC_CHUNK + (s + 1) * 512)
                        mms.append(nc.tensor.matmul(
                            psh[:, s * 512:(s + 1) * 512],
                            w1T[:, e * C:(e + 1) * C],
                            x_sb[:, rs],
                            start=True, stop=True))
                    f_sb = fpool.tile([C, NC_CHUNK], F16, tag="f")
                    if hasb1:
                        silu_ins = nc.scalar.activation(
                            f_sb[:], psh[:], act_fn, bias=b1m[:, e:e + 1])
                    else:
                        silu_ins = nc.scalar.activation(f_sb[:], psh[:], act_fn)
                    pswb = ps_m.tile([C, NC_CHUNK], F32, tag="pswb")
                    for s in range(NC_CHUNK // 512):
                        ws = slice(i * NC_CHUNK + s * 512,
                                   i * NC_CHUNK + (s + 1) * 512)
                        mms.append(nc.tensor.matmul(
                            pswb[:, s * 512:(s + 1) * 512],
                            sel[:, e * C:(e + 1) * C], wcm_sb[:, ws],
                            start=True, stop=True))
                    g_sb = gpool.tile([C, NC_CHUNK], F16, tag="g")
                    mult_ins = nc.vector.tensor_mul(g_sb[:], f_sb[:], pswb[:])
                    for s in range(NC_CHUNK // 512):
                        ss = slice(s * 512, (s + 1) * 512)
                        mms.append(nc.tensor.matmul(
                            pso[:, ss],
                            w2T[:, e * C:(e + 1) * C],
                            g_sb[:, ss],
                            start=(e == 0),
                            stop=(e == E - 1) and not hasb2))
                    # absorb all cross-engine + psum-WAW deps into PE nops
                    pe_absorb([hist_silu[0], hist_mult[-1], prev_out,
                               hist_hmm[0], hist_wb[-1], prev_l2last,
                               sel_cp if (i == 0 and e == 0) else None],
                              mms[:1])
                    for m in mms[1:]:
                        add_dep_helper(m.ins, mms[0].ins, sync=False)
                    pe_absorb([silu_ins, mult_ins], mms[-NC_CHUNK // 512:])
                    hist_silu = [hist_silu[-1], silu_ins]
                    hist_mult = [hist_mult[-1], mult_ins]
                    hist_hmm = [hist_hmm[-1], mms[1]]
                    hist_wb = [hist_wb[-1], mms[NC_CHUNK // 512 + 1]]
                    if e == E - 1:
                        prev_l2last = mms[-1]
                if hasb2:
                    for s in range(NC_CHUNK // 512):
                        ss = slice(s * 512, (s + 1) * 512)
                        rs = slice(i * NC_CHUNK + s * 512,
                                   i * NC_CHUNK + (s + 1) * 512)
                        nc.tensor.matmul(
                            pso[:, ss], b2m[:], wcm_sb[:, rs],
                            start=False, stop=True)
                # delta only — the +x residual happens on host in f32
                o_sb = opool.tile([C, NC_CHUNK], F8, tag="o")
                prev_out = nc.scalar.copy(o_sb[:], pso[:])
                nc.sync.dma_start(out_d[:, cs], o_sb[:])
            mpsum2.__exit__(None, None, None)
            mpsum.__exit__(None, None, None)
    _split_waits(nc)
    return nc


# --- host side -------------------------------------------------------------
# Replicates bass_utils.run_bass_kernel_spmd's axon execution path
# (shard_map over jax.devices()[:8] + the _bass_exec_p custom call), but
# builds the jitted callable ONCE and reuses it: re-tracing and re-lowering
# on every call costs seconds of host time while computing nothing new.
_state: dict = {}


def _get_state(key):
    if key in _state:
        return _state[key]
    install_neuronx_cc_hook()
    nc = build_kernel(*key)
    partition_name = (nc.partition_id_tensor.name
                      if nc.partition_id_tensor else None)
    in_names, out_names, out_avals = [], [], []
    for alloc in nc.m.functions[0].allocations:
        if not isinstance(alloc, mybir.MemoryLocationSet):
            continue
        name = alloc.memorylocations[0].name
        if alloc.kind == "ExternalInput":
            if name != partition_name:
                in_names.append(name)
        elif alloc.kind == "ExternalOutput":
            out_names.append(name)
            out_avals.append(jax.core.ShapedArray(
                tuple(alloc.tensor_shape), mybir.dt.np(alloc.dtype)))
    n_params = len(in_names)
    n_outs = len(out_avals)
    all_names = tuple(in_names + out_names
                      + ([partition_name] if partition_name else []))

    def _body(*args):
        operands = list(args)
        if partition_name is not None:
            operands.append(partition_id_tensor())
        outs = _bass_exec_p.bind(
            *operands,
            out_avals=tuple(out_avals),
            in_names=all_names,
            out_names=tuple(out_names),
            lowering_input_output_aliases=(),
            sim_require_finite=True,
            sim_require_nnan=True,
            nc=nc,
        )
        return tuple(outs)

    devices = jax.devices()[:NCORES]
    mesh = Mesh(np.asarray(devices), ("core",))
    sharding = NamedSharding(mesh, PartitionSpec("core"))
    donate = tuple(range(n_params, n_params + n_outs))
    njit = jax.jit(
        shard_map(_body, mesh=mesh,
                  in_specs=(PartitionSpec("core"),) * (n_params + n_outs),
                  out_specs=(PartitionSpec("core"),) * n_outs,
                  check_rep=False),
        donate_argnums=donate, keep_unused=True)
    mkdonor = jax.jit(
        lambda: jnp.zeros((NCORES * C, NS), NP_F8), out_shardings=sharding)
    st = {"nc": nc, "njit": njit, "mkdonor": mkdonor, "sharding": sharding,
          "devices": devices, "in_names": in_names, "donors": [],
          "wt_sig": None, "wt_dev": None}
    _state[key] = st
    return st


def kernel(x, gate_w, gate_b, w1, b1, w2, b2):
    x = np.asarray(x, dtype=np.float32)
    gate_w = np.asarray(gate_w, dtype=np.float32)
    gate_b = np.asarray(gate_b, dtype=np.float32)
    w1 = np.asarray(w1, dtype=np.float32)
    b1 = np.asarray(b1, dtype=np.float32)
    w2 = np.asarray(w2, dtype=np.float32)
    b2 = np.asarray(b2, dtype=np.float32)

    key = (bool(b1.any()), bool(b2.any()))
    st = _get_state(key)

    xr = x.reshape(B, C, THW)

    # x split-0 shards, quantized to fp8. Pack and upload per-shard so
    # shard c's (async) transfer overlaps packing of shard c+1, and the
    # whole upload overlaps the host gating compute below. Later splits
    # are packed after dispatching earlier ones.
    devices = st["devices"]

    def put_x(sp):
        shards = []
        for c in range(NCORES):
            bb, q = divmod(c, CPB)
            lo = q * NSH + sp * NS
            sh = xr[bb, :, lo:lo + NS].astype(NP_F8)
            shards.append(jax.device_put(sh, devices[c]))
        return jax.make_array_from_single_device_arrays(
            (NCORES * C, NS), st["sharding"], shards)

    x8_dev0 = put_x(0)

    # weights: device-resident across calls; re-uploaded only on change
    cached = st["wt_sig"]
    if (cached is None or not np.array_equal(cached[0], w1)
            or not np.array_equal(cached[1], w2)
            or not np.array_equal(cached[2], b1)):
        wt_cols = 2 * E * C + (E if key[0] else 0)
        wpk = np.empty((C, wt_cols), dtype=np.float16)
        wpk[:, 0:E * C] = w1.T
        wpk[:, E * C:2 * E * C] = w2.transpose(2, 0, 1).reshape(C, E * C)
        if key[0]:
            wpk[:, 2 * E * C:] = b1.reshape(E, C).T
        st["wt_dev"] = jax.device_put(np.tile(wpk, (NCORES, 1)),
                                      st["sharding"])
        st["wt_sig"] = (w1.copy(), w2.copy(), b1.copy())

    # --- gating on host, exact f32 ---
    logits = np.empty((B, E, THW), dtype=np.float32)
    for bb in range(B):
        np.matmul(gate_w, xr[bb], out=logits[bb])
    if gate_b.any():
        logits += gate_b[None, :, None]
    # top-2 over experts
    lt = logits.transpose(0, 2, 1).reshape(NVOX, E)
    i1 = np.argmax(lt, axis=1)
    v1 = lt[np.arange(NVOX), i1]
    lt2 = lt.copy()
    lt2[np.arange(NVOX), i1] = -np.inf
    i2 = np.argmax(lt2, axis=1)
    v2 = lt2[np.arange(NVOX), i2]
    # softmax over the two kept logits
    ez = 1.0 / (1.0 + np.exp(v2 - v1))
    wcm = np.zeros((NVOX, E), dtype=np.float32)
    wcm[np.arange(NVOX), i1] = ez
    wcm[np.arange(NVOX), i2] = 1.0 - ez
    wcm = wcm.reshape(B, THW, E).transpose(0, 2, 1)   # [B, E, THW]

    def wc_for(sp):
        wcg = np.empty((NCORES * E, NS), dtype=NP_F8)
        for c in range(NCORES):
            bb, q = divmod(c, CPB)
            lo = q * NSH + sp * NS
            wcg[c * E:(c + 1) * E] = wcm[bb, :, lo:lo + NS]
        return wcg

    b2t = (np.tile(b2.astype(np.float16), (NCORES, 1)) if key[1] else None)
    donors = st["donors"]
    st["donors"] = []

    def dispatch(sp, x8_dev):
        args_np = {"x8": x8_dev, "wc": wc_for(sp), "wt": st["wt_dev"],
                   "b2m": b2t}
        args = [args_np[n] for n in st["in_names"]]
        donor = donors.pop() if donors else st["mkdonor"]()
        return st["njit"](*args, donor)[0]

    outs = [None] * NSPLIT
    outs[0] = dispatch(0, x8_dev0)
    for sp in range(1, NSPLIT):
        outs[sp] = dispatch(sp, put_x(sp))

    # fetch the fp8 deltas per shard (kick all host copies off async, in
    # completion order) and interleave the f32 residual adds with the
    # remaining transfers
    all_shards = []
    for sp in range(NSPLIT):
        shards = sorted(outs[sp].addressable_shards,
                        key=lambda s: s.index[0].start or 0)
        all_shards.append(shards)
        for s in shards:
            try:
                s.data.copy_to_host_async()
            except Exception:
                pass
    out = np.empty((B, C, THW), dtype=np.float32)
    for sp in range(NSPLIT):
        for s in all_shards[sp]:
            c = (s.index[0].start or 0) // C
            bb, q = divmod(c, CPB)
            lo = q * NSH + sp * NS
            np.add(xr[bb, :, lo:lo + NS], np.asarray(s.data),
                   out=out[bb, :, lo:lo + NS])
        st["donors"].append(outs[sp])   # recycle as future donations
    return out.reshape(B, C, T, H, W)


# revision 24
# speedup vs baseline: 6.4166x; 1.0764x over previous
import sys

sys.path.insert(0, "/opt/trn_rl_repo")

import numpy as np

import jax
import jax.numpy as jnp
from jax.sharding import Mesh, NamedSharding, PartitionSpec
from jax.experimental.shard_map import shard_map

import concourse.bass as bass
import concourse.tile as tile
from concourse import mybir
from concourse.tile import add_dep_helper
from concourse.bass2jax import (
    _bass_exec_p, install_neuronx_cc_hook, partition_id_tensor)

# Problem constants (nn_MoEBlock: B,C,T,H,W = 2,128,8,64,64; E=8; top-2)
B, C, T, H, W = 2, 128, 8, 64, 64
E = 8
THW = T * H * W               # 32768
NVOX = B * THW                # 65536 voxels
NCORES = 8
NSH = NVOX // NCORES          # 8192 voxels per core
CPB = NCORES // B             # cores per batch element (4)
NSPLIT = 2                    # pipeline depth: device exec and both wire
                              # directions overlap across splits (the
                              # per-device queue is FIFO in issue order,
                              # but up- and down-transfers are full duplex)
NS = NSH // NSPLIT            # voxels per core per call
NC_CHUNK = 1024               # main-loop chunk (voxels)
NCHUNKS = NS // NC_CHUNK
F16 = mybir.dt.float16
F32 = mybir.dt.float32
F8 = mybir.dt.float8e4
NP_F8 = mybir.dt.np(F8)       # ml_dtypes.float8_e4m3


def _split_waits(nc, max_waits=1):
    """The walrus scheduler accepts only one sync-wait per instruction.
    Move extra on_wait conditions onto standalone same-engine NoOps
    inserted immediately before the instruction (same engine stream =>
    identical semantics)."""
    ctr = 0
    for f in nc.m.functions:
        for bb in f.blocks:
            insts = list(bb.instructions)
            out = []
            changed = False
            for inst in insts:
                si = inst.sync_info
                w = list(si.on_wait) if si is not None and si.on_wait else []
                if (len(w) > max_waits
                        and inst.engine != mybir.EngineType.Unassigned):
                    for extra in w[:-max_waits]:
                        ctr += 1
                        nop = mybir.InstNoOp(
                            name=f"WSPLIT-{ctr}", ins=[], outs=[])
                        nop.engine = inst.engine
                        nop.sync_info = mybir.SyncInfo(
                            on_wait=[extra], on_update=[])
                        out.append(nop)
                    inst.sync_info = mybir.SyncInfo(
                        on_wait=w[-max_waits:],
                        on_update=list(si.on_update) if si.on_update else [])
                    changed = True
                out.append(inst)
            if changed:
                try:
                    bb.instructions = out
                except Exception:
                    bb.instructions.clear()
                    bb.instructions.extend(out)
    return nc


def build_kernel(hasb1: bool, hasb2: bool):
    """Expert MLP only — gating (logits/top-2/softmax) happens on host in
    exact f32; the per-voxel expert weights arrive as the f8 `wc` input.
    Output is the MoE delta (no +x residual; host adds it in f32)."""
    act_fn = mybir.ActivationFunctionType.Silu
    wt_cols = 2 * E * C + (E if hasb1 else 0)
    nc = bass.Bass()
    x8_d = nc.dram_tensor("x8", [C, NS], F8, kind="ExternalInput")
    wc_d = nc.dram_tensor("wc", [E, NS], F8, kind="ExternalInput")
    wt_d = nc.dram_tensor("wt", [C, wt_cols], F16, kind="ExternalInput")
    b2_d = (nc.dram_tensor("b2m", [E, C], F16, kind="ExternalInput")
            if hasb2 else None)
    out_d = nc.dram_tensor("out", [C, NS], F8, kind="ExternalOutput")

    with tile.TileContext(nc) as tc:
        with (
            tc.tile_pool(name="consts", bufs=1) as consts,
            tc.tile_pool(name="xp", bufs=1) as xp,
            tc.tile_pool(name="fpool", bufs=3) as fpool,
            tc.tile_pool(name="gpool", bufs=3) as gpool,
            tc.tile_pool(name="opool", bufs=2) as opool,
        ):
            # ---------- loads ----------
            x_sb = xp.tile([C, NS], F8)
            wcm_sb = consts.tile([E, NS], F8)
            w1T = consts.tile([C, E * C], F16)
            w2T = consts.tile([C, E * C], F16)
            sel32 = consts.tile([E, E * C], F32)
            sel = consts.tile([E, E * C], F16)

            dmas = []
            for j in range(4):
                s = slice(j * (NS // 4), (j + 1) * (NS // 4))
                dmas.append(nc.sync.dma_start(x_sb[:, s], x8_d[:, s]))
            dmas.append(nc.sync.dma_start(wcm_sb[:], wc_d[:]))
            dmas.append(nc.sync.dma_start(w1T[:], wt_d[:, 0:E * C]))
            dmas.append(nc.sync.dma_start(w2T[:], wt_d[:, E * C:2 * E * C]))
            if hasb1:
                b1m = consts.tile([C, E], F32)
                bh = consts.tile([C, E], F16)
                dmas.append(nc.sync.dma_start(
                    bh[:], wt_d[:, 2 * E * C:2 * E * C + E]))
                nc.scalar.copy(b1m[:], bh[:])
            if hasb2:
                b2m = consts.tile([E, C], F16)
                dmas.append(nc.sync.dma_start(b2m[:], b2_d[:]))

            # on-device constant: expert-broadcast selector
            # sel[e, e*C:(e+1)*C] = 1, else 0
            nc.vector.memset(sel32[:], 1.0)
            nc.gpsimd.affine_select(
                sel32[:], sel32[:], pattern=[[1, E * C]], base=0,
                channel_multiplier=-C,
                compare_op=mybir.AluOpType.is_ge, fill=0.0)
            nc.gpsimd.affine_select(
                sel32[:], sel32[:], pattern=[[-1, E * C]], base=C - 1,
                channel_multiplier=C,
                compare_op=mybir.AluOpType.is_ge, fill=0.0)
            sel_cp = nc.scalar.copy(sel[:], sel32[:])

            # PE carries only ONE sync wait per Matmult through walrus;
            # absorb each input-DMA dependency into a PE nop up front.
            dma_nops = []
            for dma in dmas:
                nop = nc.tensor.nop(nofuse=True)
                add_dep_helper(nop.ins, dma.ins, sync=True)
                dma_nops.append(nop)

            def pe_absorb(producers, consumer_mms):
                nops = []
                for p in producers:
                    if p is None:
                        continue
                    n = nc.tensor.nop(nofuse=True)
                    add_dep_helper(n.ins, p.ins, sync=True)
                    nops.append(n)
                for m in consumer_mms:
                    for n in nops:
                        add_dep_helper(m.ins, n.ins, sync=False)

            # ---------- experts + combine ----------
            mpsum = tc.tile_pool(name="ps_m", bufs=1, space="PSUM")
            ps_m = mpsum.__enter__()
            mpsum2 = tc.tile_pool(name="ps_m2", bufs=2, space="PSUM")
            ps_m2 = mpsum2.__enter__()
            prev_out = prev_l2last = None
            hist_silu = [None, None]
            hist_mult = [None, None]
            hist_hmm = [None, None]
            hist_wb = [None, None]
            for i in range(NCHUNKS):
                cs = slice(i * NC_CHUNK, (i + 1) * NC_CHUNK)
                pso = ps_m.tile([C, NC_CHUNK], F32, tag="pso")
                for e in range(E):
                    mms = []
                    psh = ps_m2.tile([C, NC_CHUNK], F32, tag="psh")
                    for s in range(NC_CHUNK // 512):
                        rs = slice(i * NC_CHUNK + s * 512,
                                   i * NC_CHUNK + (s + 1) * 512)
                        mms.append(nc.tensor.matmul(
                            psh[:, s * 512:(s + 1) * 512],
                            w1T[:, e * C:(e + 1) * C],
                            x_sb[:, rs],
                            start=True, stop=True))
                    f_sb = fpool.tile([C, NC_CHUNK], F16, tag="f")
                    if hasb1:
                        silu_ins = nc.scalar.activation(
                            f_sb[:], psh[:], act_fn, bias=b1m[:, e:e + 1])
                    else:
                        silu_ins = nc.scalar.activation(f_sb[:], psh[:], act_fn)
                    pswb = ps_m.tile([C, NC_CHUNK], F32, tag="pswb")
                    for s in range(NC_CHUNK // 512):
                        ws = slice(i * NC_CHUNK + s * 512,
                                   i * NC_CHUNK + (s + 1) * 512)
                        mms.append(nc.tensor.matmul(
                            pswb[:, s * 512:(s + 1) * 512],
                            sel[:, e * C:(e + 1) * C], wcm_sb[:, ws],
                            start=True, stop=True))
                    g_sb = gpool.tile([C, NC_CHUNK], F16, tag="g")
                    mult_ins = nc.vector.tensor_mul(g_sb[:], f_sb[:], pswb[:])
                    for s in range(NC_CHUNK // 512):
                        ss = slice(s * 512, (s + 1) * 512)
                        mms.append(nc.tensor.matmul(
                            pso[:, ss],
                            w2T[:, e * C:(e + 1) * C],
                            g_sb[:, ss],
                            start=(e == 0),
                            stop=(e == E - 1) and not hasb2))
                    # absorb all cross-engine + psum-WAW deps into PE nops
                    pe_absorb([hist_silu[0], hist_mult[-1], prev_out,
                               hist_hmm[0], hist_wb[-1], prev_l2last,
                               sel_cp if (i == 0 and e == 0) else None],
                              mms[:1])
                    for m in mms[1:]:
                        add_dep_helper(m.ins, mms[0].ins, sync=False)
                    pe_absorb([silu_ins, mult_ins], mms[-NC_CHUNK // 512:])
                    hist_silu = [hist_silu[-1], silu_ins]
                    hist_mult = [hist_mult[-1], mult_ins]
                    hist_hmm = [hist_hmm[-1], mms[1]]
                    hist_wb = [hist_wb[-1], mms[NC_CHUNK // 512 + 1]]
                    if e == E - 1:
                        prev_l2last = mms[-1]
                if hasb2:
                    for s in range(NC_CHUNK // 512):
                        ss = slice(s * 512, (s + 1) * 512)
                        rs = slice(i * NC_CHUNK + s * 512,
                                   i * NC_CHUNK + (s + 1) * 512)
                        nc.tensor.matmul(
                            pso[:, ss], b2m[:], wcm_sb[:, rs],
                            start=False, stop=True)
                # delta only — the +x residual happens on host in f32
                o_sb = opool.tile([C, NC_CHUNK], F8, tag="o")
                prev_out = nc.scalar.copy(o_sb[:], pso[:])
                nc.sync.dma_start(out_d[:, cs], o_sb[:])
            mpsum2.__exit__(None, None, None)
            mpsum.__exit__(None, None, None)
    _split_waits(nc)
    return nc


# --- host side -------------------------------------------------------------
# Replicates bass_utils.run_bass_kernel_spmd's axon execution path
# (shard_map over jax.devices()[:8] + the _bass_exec_p custom call), but
# builds the jitted callable ONCE and reuses it: re-tracing and re-lowering
# on every call costs seconds of host time while computing nothing new.
_state: dict = {}


def _get_state(key):
    if key in _state:
        return _state[key]
    install_neuronx_cc_hook()
    nc = build_kernel(*key)
    partition_name = (nc.partition_id_tensor.name
                      if nc.partition_id_tensor else None)
    in_names, out_names, out_avals = [], [], []
    for alloc in nc.m.functions[0].allocations:
        if not isinstance(alloc, mybir.MemoryLocationSet):
            continue
        name = alloc.memorylocations[0].name
        if alloc.kind == "ExternalInput":
            if name != partition_name:
                in_names.append(name)
        elif alloc.kind == "ExternalOutput":
            out_names.append(name)
            out_avals.append(jax.core.ShapedArray(
                tuple(alloc.tensor_shape), mybir.dt.np(alloc.dtype)))
    n_params = len(in_names)
    n_outs = len(out_avals)
    all_names = tuple(in_names + out_names
                      + ([partition_name] if partition_name else []))

    def _body(*args):
        operands = list(args)
        if partition_name is not None:
            operands.append(partition_id_tensor())
        outs = _bass_exec_p.bind(
            *operands,
            out_avals=tuple(out_avals),
            in_names=all_names,
            out_names=tuple(out_names),
            lowering_input_output_aliases=(),
            sim_require_finite=True,
            sim_require_nnan=True,
            nc=nc,
        )
        return tuple(outs)

    devices = jax.devices()[:NCORES]
    mesh = Mesh(np.asarray(devices), ("core",))
    sharding = NamedSharding(mesh, PartitionSpec("core"))
    donate = tuple(range(n_params, n_params + n_outs))
    njit = jax.jit(
        shard_map(_body, mesh=mesh,
                  in_specs=(PartitionSpec("core"),) * (n_params + n_outs),
                  out_specs=(PartitionSpec("core"),) * n_outs,
                  check_rep=False),
        donate_argnums=donate, keep_unused=True)
    mkdonor = jax.jit(
        lambda: jnp.zeros((NCORES * C, NS), NP_F8), out_shardings=sharding)
    st = {"nc": nc, "njit": njit, "mkdonor": mkdonor, "sharding": sharding,
          "devices": devices, "in_names": in_names, "donors": [],
          "wt_sig": None, "wt_dev": None}
    _state[key] = st
    return st


def kernel(x, gate_w, gate_b, w1, b1, w2, b2):
    x = np.asarray(x, dtype=np.float32)
    gate_w = np.asarray(gate_w, dtype=np.float32)
    gate_b = np.asarray(gate_b, dtype=np.float32)
    w1 = np.asarray(w1, dtype=np.float32)
    b1 = np.asarray(b1, dtype=np.float32)
    w2 = np.asarray(w2, dtype=np.float32)
    b2 = np.asarray(b2, dtype=np.float32)

    key = (bool(b1.any()), bool(b2.any()))
    st = _get_state(key)

    xr = x.reshape(B, C, THW)

    # x split-0 shards, quantized to fp8. Pack and upload per-shard so
    # shard c's (async) transfer overlaps packing of shard c+1, and the
    # whole upload overlaps the host gating compute below. Later splits
    # are packed after dispatching earlier ones.
    devices = st["devices"]

    def put_x(sp):
        shards = []
        for c in range(NCORES):
            bb, q = divmod(c, CPB)
            lo = q * NSH + sp * NS
            sh = xr[bb, :, lo:lo + NS].astype(NP_F8)
            shards.append(jax.device_put(sh, devices[c]))
        return jax.make_array_from_single_device_arrays(
            (NCORES * C, NS), st["sharding"], shards)

    x8_dev0 = put_x(0)   # upload in flight during the host work below

    # weights: device-resident across calls; re-uploaded only on change
    cached = st["wt_sig"]
    if (cached is None or not np.array_equal(cached[0], w1)
            or not np.array_equal(cached[1], w2)
            or not np.array_equal(cached[2], b1)):
        wt_cols = 2 * E * C + (E if key[0] else 0)
        wpk = np.empty((C, wt_cols), dtype=np.float16)
        wpk[:, 0:E * C] = w1.T
        wpk[:, E * C:2 * E * C] = w2.transpose(2, 0, 1).reshape(C, E * C)
        if key[0]:
            wpk[:, 2 * E * C:] = b1.reshape(E, C).T
        st["wt_dev"] = jax.device_put(np.tile(wpk, (NCORES, 1)),
                                      st["sharding"])
        st["wt_sig"] = (w1.copy(), w2.copy(), b1.copy())

    # --- gating on host, exact f32, one split at a time ---
    xr4 = x.reshape(B, C, CPB, NSH)
    has_gb = bool(gate_b.any())

    def gate_split(sp):
        lo = sp * NS
        xs = xr4[:, :, :, lo:lo + NS]                  # [B, C, CPB, NS] view
        lg = np.tensordot(gate_w, xs, axes=(1, 1))     # [E, B, CPB, NS]
        if has_gb:
            lg += gate_b[:, None, None, None]
        lt = lg.reshape(E, -1)
        n2 = lt.shape[1]
        ar = np.arange(n2)
        i1 = np.argmax(lt, axis=0)
        v1 = lt[i1, ar]
        lt[i1, ar] = -np.inf
        i2 = np.argmax(lt, axis=0)
        v2 = lt[i2, ar]
        ez = 1.0 / (1.0 + np.exp(v2 - v1))             # top-2 softmax
        w32 = np.zeros((E, n2), dtype=np.float32)
        w32[i1, ar] = ez
        w32[i2, ar] = 1.0 - ez
        return (w32.reshape(E, B, CPB, NS).transpose(1, 2, 0, 3)
                .astype(NP_F8).reshape(NCORES * E, NS))

    b2t = (np.tile(b2.astype(np.float16), (NCORES, 1)) if key[1] else None)
    donors = st["donors"]
    st["donors"] = []

    def dispatch(sp, x8_dev):
        args_np = {"x8": x8_dev, "wc": gate_split(sp), "wt": st["wt_dev"],
                   "b2m": b2t}
        args = [args_np[n] for n in st["in_names"]]
        donor = donors.pop() if donors else st["mkdonor"]()
        return st["njit"](*args, donor)[0]

    # Issue order is the device queue order: dispatch split sp and kick
    # its result-fetch asyncs (pre-queued, so the data streams back the
    # moment exec sp finishes) BEFORE uploading split sp+1 — the fetch of
    # sp overlaps the upload of sp+1 on the full-duplex link.
    outs = [None] * NSPLIT
    all_shards = [None] * NSPLIT
    for sp in range(NSPLIT):
        outs[sp] = dispatch(sp, x8_dev0 if sp == 0 else put_x(sp))
        shards = sorted(outs[sp].addressable_shards,
                        key=lambda s: s.index[0].start or 0)
        all_shards[sp] = shards
        for s in shards:
            try:
                s.data.copy_to_host_async()
            except Exception:
                pass
    # interleave the f32 residual adds with the remaining transfers
    out = np.empty((B, C, THW), dtype=np.float32)
    for sp in range(NSPLIT):
        for s in all_shards[sp]:
            c = (s.index[0].start or 0) // C
            bb, q = divmod(c, CPB)
            lo = q * NSH + sp * NS
            np.add(xr[bb, :, lo:lo + NS], np.asarray(s.data),
                   out=out[bb, :, lo:lo + NS])
        st["donors"].append(outs[sp])   # recycle as future donations
    return out.reshape(B, C, T, H, W)


# revision 25
# speedup vs baseline: 7.5214x; 1.1722x over previous
import sys

sys.path.insert(0, "/opt/trn_rl_repo")

import numpy as np

import jax
import jax.numpy as jnp
from jax.sharding import Mesh, NamedSharding, PartitionSpec
from jax.experimental.shard_map import shard_map

import concourse.bass as bass
import concourse.tile as tile
from concourse import mybir
from concourse.tile import add_dep_helper
from concourse.bass2jax import (
    _bass_exec_p, install_neuronx_cc_hook, partition_id_tensor)

# Problem constants (nn_MoEBlock: B,C,T,H,W = 2,128,8,64,64; E=8; top-2)
B, C, T, H, W = 2, 128, 8, 64, 64
E = 8
THW = T * H * W               # 32768
NVOX = B * THW                # 65536 voxels
NCORES = 8
NSH = NVOX // NCORES          # 8192 voxels per core
CPB = NCORES // B             # cores per batch element (4)
NSPLIT = 2                    # pipeline depth: device exec and both wire
                              # directions overlap across splits (the
                              # per-device queue is FIFO in issue order,
                              # but up- and down-transfers are full duplex)
NS = NSH // NSPLIT            # voxels per core per call
NC_CHUNK = 1024               # main-loop chunk (voxels)
NCHUNKS = NS // NC_CHUNK
F16 = mybir.dt.float16
F32 = mybir.dt.float32
F8 = mybir.dt.float8e4
NP_F8 = mybir.dt.np(F8)       # ml_dtypes.float8_e4m3


def _split_waits(nc, max_waits=1):
    """The walrus scheduler accepts only one sync-wait per instruction.
    Move extra on_wait conditions onto standalone same-engine NoOps
    inserted immediately before the instruction (same engine stream =>
    identical semantics)."""
    ctr = 0
    for f in nc.m.functions:
        for bb in f.blocks:
            insts = list(bb.instructions)
            out = []
            changed = False
            for inst in insts:
                si = inst.sync_info
                w = list(si.on_wait) if si is not None and si.on_wait else []
                if (len(w) > max_waits
                        and inst.engine != mybir.EngineType.Unassigned):
                    for extra in w[:-max_waits]:
                        ctr += 1
                        nop = mybir.InstNoOp(
                            name=f"WSPLIT-{ctr}", ins=[], outs=[])
                        nop.engine = inst.engine
                        nop.sync_info = mybir.SyncInfo(
                            on_wait=[extra], on_update=[])
                        out.append(nop)
                    inst.sync_info = mybir.SyncInfo(
                        on_wait=w[-max_waits:],
                        on_update=list(si.on_update) if si.on_update else [])
                    changed = True
                out.append(inst)
            if changed:
                try:
                    bb.instructions = out
                except Exception:
                    bb.instructions.clear()
                    bb.instructions.extend(out)
    return nc


def build_kernel(hasb1: bool, hasb2: bool):
    """Expert MLP only — gating (logits/top-2/softmax) happens on host in
    exact f32; the per-voxel expert weights arrive as the f8 `wc` input.
    Output is the MoE delta (no +x residual; host adds it in f32)."""
    act_fn = mybir.ActivationFunctionType.Silu
    wt_cols = 2 * E * C + (E if hasb1 else 0)
    nc = bass.Bass()
    x8_d = nc.dram_tensor("x8", [C, NS], F8, kind="ExternalInput")
    wc_d = nc.dram_tensor("wc", [E, NS], F8, kind="ExternalInput")
    wt_d = nc.dram_tensor("wt", [C, wt_cols], F16, kind="ExternalInput")
    b2_d = (nc.dram_tensor("b2m", [E, C], F16, kind="ExternalInput")
            if hasb2 else None)
    out_d = nc.dram_tensor("out", [C, NS], F8, kind="ExternalOutput")

    with tile.TileContext(nc) as tc:
        with (
            tc.tile_pool(name="consts", bufs=1) as consts,
            tc.tile_pool(name="xp", bufs=1) as xp,
            tc.tile_pool(name="fpool", bufs=3) as fpool,
            tc.tile_pool(name="gpool", bufs=3) as gpool,
            tc.tile_pool(name="opool", bufs=2) as opool,
        ):
            # ---------- loads ----------
            x_sb = xp.tile([C, NS], F8)
            wcm_sb = consts.tile([E, NS], F8)
            w1T = consts.tile([C, E * C], F16)
            w2T = consts.tile([C, E * C], F16)
            sel32 = consts.tile([E, E * C], F32)
            sel = consts.tile([E, E * C], F16)

            dmas = []
            for j in range(4):
                s = slice(j * (NS // 4), (j + 1) * (NS // 4))
                dmas.append(nc.sync.dma_start(x_sb[:, s], x8_d[:, s]))
            dmas.append(nc.sync.dma_start(wcm_sb[:], wc_d[:]))
            dmas.append(nc.sync.dma_start(w1T[:], wt_d[:, 0:E * C]))
            dmas.append(nc.sync.dma_start(w2T[:], wt_d[:, E * C:2 * E * C]))
            if hasb1:
                b1m = consts.tile([C, E], F32)
                bh = consts.tile([C, E], F16)
                dmas.append(nc.sync.dma_start(
                    bh[:], wt_d[:, 2 * E * C:2 * E * C + E]))
                nc.scalar.copy(b1m[:], bh[:])
            if hasb2:
                b2m = consts.tile([E, C], F16)
                dmas.append(nc.sync.dma_start(b2m[:], b2_d[:]))

            # on-device constant: expert-broadcast selector
            # sel[e, e*C:(e+1)*C] = 1, else 0
            nc.vector.memset(sel32[:], 1.0)
            nc.gpsimd.affine_select(
                sel32[:], sel32[:], pattern=[[1, E * C]], base=0,
                channel_multiplier=-C,
                compare_op=mybir.AluOpType.is_ge, fill=0.0)
            nc.gpsimd.affine_select(
                sel32[:], sel32[:], pattern=[[-1, E * C]], base=C - 1,
                channel_multiplier=C,
                compare_op=mybir.AluOpType.is_ge, fill=0.0)
            sel_cp = nc.scalar.copy(sel[:], sel32[:])

            # PE carries only ONE sync wait per Matmult through walrus;
            # absorb each input-DMA dependency into a PE nop up front.
            dma_nops = []
            for dma in dmas:
                nop = nc.tensor.nop(nofuse=True)
                add_dep_helper(nop.ins, dma.ins, sync=True)
                dma_nops.append(nop)

            def pe_absorb(producers, consumer_mms):
                nops = []
                for p in producers:
                    if p is None:
                        continue
                    n = nc.tensor.nop(nofuse=True)
                    add_dep_helper(n.ins, p.ins, sync=True)
                    nops.append(n)
                for m in consumer_mms:
                    for n in nops:
                        add_dep_helper(m.ins, n.ins, sync=False)

            # ---------- experts + combine ----------
            mpsum = tc.tile_pool(name="ps_m", bufs=1, space="PSUM")
            ps_m = mpsum.__enter__()
            mpsum2 = tc.tile_pool(name="ps_m2", bufs=2, space="PSUM")
            ps_m2 = mpsum2.__enter__()
            prev_out = prev_l2last = None
            hist_silu = [None, None]
            hist_mult = [None, None]
            hist_hmm = [None, None]
            hist_wb = [None, None]
            for i in range(NCHUNKS):
                cs = slice(i * NC_CHUNK, (i + 1) * NC_CHUNK)
                pso = ps_m.tile([C, NC_CHUNK], F32, tag="pso")
                for e in range(E):
                    mms = []
                    psh = ps_m2.tile([C, NC_CHUNK], F32, tag="psh")
                    for s in range(NC_CHUNK // 512):
                        rs = slice(i * NC_CHUNK + s * 512,
                                   i * NC_CHUNK + (s + 1) * 512)
                        mms.append(nc.tensor.matmul(
                            psh[:, s * 512:(s + 1) * 512],
                            w1T[:, e * C:(e + 1) * C],
                            x_sb[:, rs],
                            start=True, stop=True))
                    f_sb = fpool.tile([C, NC_CHUNK], F16, tag="f")
                    if hasb1:
                        silu_ins = nc.scalar.activation(
                            f_sb[:], psh[:], act_fn, bias=b1m[:, e:e + 1])
                    else:
                        silu_ins = nc.scalar.activation(f_sb[:], psh[:], act_fn)
                    pswb = ps_m.tile([C, NC_CHUNK], F32, tag="pswb")
                    for s in range(NC_CHUNK // 512):
                        ws = slice(i * NC_CHUNK + s * 512,
                                   i * NC_CHUNK + (s + 1) * 512)
                        mms.append(nc.tensor.matmul(
                            pswb[:, s * 512:(s + 1) * 512],
                            sel[:, e * C:(e + 1) * C], wcm_sb[:, ws],
                            start=True, stop=True))
                    g_sb = gpool.tile([C, NC_CHUNK], F16, tag="g")
                    mult_ins = nc.vector.tensor_mul(g_sb[:], f_sb[:], pswb[:])
                    for s in range(NC_CHUNK // 512):
                        ss = slice(s * 512, (s + 1) * 512)
                        mms.append(nc.tensor.matmul(
                            pso[:, ss],
                            w2T[:, e * C:(e + 1) * C],
                            g_sb[:, ss],
                            start=(e == 0),
                            stop=(e == E - 1) and not hasb2))
                    # absorb all cross-engine + psum-WAW deps into PE nops
                    pe_absorb([hist_silu[0], hist_mult[-1], prev_out,
                               hist_hmm[0], hist_wb[-1], prev_l2last,
                               sel_cp if (i == 0 and e == 0) else None],
                              mms[:1])
                    for m in mms[1:]:
                        add_dep_helper(m.ins, mms[0].ins, sync=False)
                    pe_absorb([silu_ins, mult_ins], mms[-NC_CHUNK // 512:])
                    hist_silu = [hist_silu[-1], silu_ins]
                    hist_mult = [hist_mult[-1], mult_ins]
                    hist_hmm = [hist_hmm[-1], mms[1]]
                    hist_wb = [hist_wb[-1], mms[NC_CHUNK // 512 + 1]]
                    if e == E - 1:
                        prev_l2last = mms[-1]
                if hasb2:
                    for s in range(NC_CHUNK // 512):
                        ss = slice(s * 512, (s + 1) * 512)
                        rs = slice(i * NC_CHUNK + s * 512,
                                   i * NC_CHUNK + (s + 1) * 512)
                        nc.tensor.matmul(
                            pso[:, ss], b2m[:], wcm_sb[:, rs],
                            start=False, stop=True)
                # delta only — the +x residual happens on host in f32
                o_sb = opool.tile([C, NC_CHUNK], F8, tag="o")
                prev_out = nc.scalar.copy(o_sb[:], pso[:])
                nc.sync.dma_start(out_d[:, cs], o_sb[:])
            mpsum2.__exit__(None, None, None)
            mpsum.__exit__(None, None, None)
    _split_waits(nc)
    return nc


# --- host side -------------------------------------------------------------
# Replicates bass_utils.run_bass_kernel_spmd's axon execution path
# (shard_map over jax.devices()[:8] + the _bass_exec_p custom call), but
# builds the jitted callable ONCE and reuses it: re-tracing and re-lowering
# on every call costs seconds of host time while computing nothing new.
_state: dict = {}


def _get_state(key):
    if key in _state:
        return _state[key]
    install_neuronx_cc_hook()
    nc = build_kernel(*key)
    partition_name = (nc.partition_id_tensor.name
                      if nc.partition_id_tensor else None)
    in_names, out_names, out_avals = [], [], []
    for alloc in nc.m.functions[0].allocations:
        if not isinstance(alloc, mybir.MemoryLocationSet):
            continue
        name = alloc.memorylocations[0].name
        if alloc.kind == "ExternalInput":
            if name != partition_name:
                in_names.append(name)
        elif alloc.kind == "ExternalOutput":
            out_names.append(name)
            out_avals.append(jax.core.ShapedArray(
                tuple(alloc.tensor_shape), mybir.dt.np(alloc.dtype)))
    n_params = len(in_names)
    n_outs = len(out_avals)
    all_names = tuple(in_names + out_names
                      + ([partition_name] if partition_name else []))

    def _body(*args):
        operands = list(args)
        if partition_name is not None:
            operands.append(partition_id_tensor())
        outs = _bass_exec_p.bind(
            *operands,
            out_avals=tuple(out_avals),
            in_names=all_names,
            out_names=tuple(out_names),
            lowering_input_output_aliases=(),
            sim_require_finite=True,
            sim_require_nnan=True,
            nc=nc,
        )
        return tuple(outs)

    devices = jax.devices()[:NCORES]
    mesh = Mesh(np.asarray(devices), ("core",))
    sharding = NamedSharding(mesh, PartitionSpec("core"))
    donate = tuple(range(n_params, n_params + n_outs))
    njit = jax.jit(
        shard_map(_body, mesh=mesh,
                  in_specs=(PartitionSpec("core"),) * (n_params + n_outs),
                  out_specs=(PartitionSpec("core"),) * n_outs,
                  check_rep=False),
        donate_argnums=donate, keep_unused=True)
    mkdonor = jax.jit(
        lambda: jnp.zeros((NCORES * C, NS), NP_F8), out_shardings=sharding)
    st = {"nc": nc, "njit": njit, "mkdonor": mkdonor, "sharding": sharding,
          "devices": devices, "in_names": in_names, "donors": [],
          "wt_sig": None, "wt_dev": None}
    _state[key] = st
    return st


def kernel(x, gate_w, gate_b, w1, b1, w2, b2):
    x = np.asarray(x, dtype=np.float32)
    gate_w = np.asarray(gate_w, dtype=np.float32)
    gate_b = np.asarray(gate_b, dtype=np.float32)
    w1 = np.asarray(w1, dtype=np.float32)
    b1 = np.asarray(b1, dtype=np.float32)
    w2 = np.asarray(w2, dtype=np.float32)
    b2 = np.asarray(b2, dtype=np.float32)

    key = (bool(b1.any()), bool(b2.any()))
    st = _get_state(key)

    xr = x.reshape(B, C, THW)

    # x split shards, quantized to fp8, shipped as one async device_put
    # (a single put returns ~40ms sooner than 8 per-device puts on this
    # 1-CPU host, and the upload wire time is identical). The transfer is
    # in flight during the host gating compute below.
    def put_x(sp):
        xg = np.empty((NCORES * C, NS), dtype=NP_F8)
        for c in range(NCORES):
            bb, q = divmod(c, CPB)
            lo = q * NSH + sp * NS
            xg[c * C:(c + 1) * C] = xr[bb, :, lo:lo + NS]
        return jax.device_put(xg, st["sharding"])

    x8_dev0 = put_x(0)   # upload in flight during the host work below

    # weights: device-resident across calls; re-uploaded only on change
    cached = st["wt_sig"]
    if (cached is None or not np.array_equal(cached[0], w1)
            or not np.array_equal(cached[1], w2)
            or not np.array_equal(cached[2], b1)):
        wt_cols = 2 * E * C + (E if key[0] else 0)
        wpk = np.empty((C, wt_cols), dtype=np.float16)
        wpk[:, 0:E * C] = w1.T
        wpk[:, E * C:2 * E * C] = w2.transpose(2, 0, 1).reshape(C, E * C)
        if key[0]:
            wpk[:, 2 * E * C:] = b1.reshape(E, C).T
        st["wt_dev"] = jax.device_put(np.tile(wpk, (NCORES, 1)),
                                      st["sharding"])
        st["wt_sig"] = (w1.copy(), w2.copy(), b1.copy())

    # --- gating on host, exact f32, one split at a time ---
    xr4 = x.reshape(B, C, CPB, NSH)
    has_gb = bool(gate_b.any())

    def gate_split(sp):
        lo = sp * NS
        xs = xr4[:, :, :, lo:lo + NS]                  # [B, C, CPB, NS] view
        lg = np.tensordot(gate_w, xs, axes=(1, 1))     # [E, B, CPB, NS]
        if has_gb:
            lg += gate_b[:, None, None, None]
        lt = lg.reshape(E, -1)
        n2 = lt.shape[1]
        ar = np.arange(n2)
        i1 = np.argmax(lt, axis=0)
        v1 = lt[i1, ar]
        lt[i1, ar] = -np.inf
        i2 = np.argmax(lt, axis=0)
        v2 = lt[i2, ar]
        ez = 1.0 / (1.0 + np.exp(v2 - v1))             # top-2 softmax
        w32 = np.zeros((E, n2), dtype=np.float32)
        w32[i1, ar] = ez
        w32[i2, ar] = 1.0 - ez
        return (w32.reshape(E, B, CPB, NS).transpose(1, 2, 0, 3)
                .astype(NP_F8).reshape(NCORES * E, NS))

    b2t = (np.tile(b2.astype(np.float16), (NCORES, 1)) if key[1] else None)
    donors = st["donors"]
    st["donors"] = []

    def dispatch(sp, x8_dev):
        args_np = {"x8": x8_dev, "wc": gate_split(sp), "wt": st["wt_dev"],
                   "b2m": b2t}
        args = [args_np[n] for n in st["in_names"]]
        donor = donors.pop() if donors else st["mkdonor"]()
        return st["njit"](*args, donor)[0]

    # Issue order is the device queue order: dispatch split sp and kick
    # its result-fetch asyncs (pre-queued, so the data streams back the
    # moment exec sp finishes) BEFORE uploading split sp+1 — the fetch of
    # sp overlaps the upload of sp+1 on the full-duplex link.
    outs = [None] * NSPLIT
    all_shards = [None] * NSPLIT
    for sp in range(NSPLIT):
        outs[sp] = dispatch(sp, x8_dev0 if sp == 0 else put_x(sp))
        shards = sorted(outs[sp].addressable_shards,
                        key=lambda s: s.index[0].start or 0)
        all_shards[sp] = shards
        for s in shards:
            try:
                s.data.copy_to_host_async()
            except Exception:
                pass
    # interleave the f32 residual adds with the remaining transfers
    out = np.empty((B, C, THW), dtype=np.float32)
    for sp in range(NSPLIT):
        for s in all_shards[sp]:
            c = (s.index[0].start or 0) // C
            bb, q = divmod(c, CPB)
            lo = q * NSH + sp * NS
            np.add(xr[bb, :, lo:lo + NS], np.asarray(s.data),
                   out=out[bb, :, lo:lo + NS])
        st["donors"].append(outs[sp])   # recycle as future donations
    return out.reshape(B, C, T, H, W)


# revision 35
# speedup vs baseline: 8.4207x; 1.1196x over previous
import sys

sys.path.insert(0, "/opt/trn_rl_repo")

import numpy as np

import jax
import jax.numpy as jnp
from jax.sharding import Mesh, NamedSharding, PartitionSpec
from jax.experimental.shard_map import shard_map

import concourse.bass as bass
import concourse.tile as tile
from concourse import mybir
from concourse.tile import add_dep_helper
from concourse.bass2jax import (
    _bass_exec_p, install_neuronx_cc_hook, partition_id_tensor)

# Problem constants (nn_MoEBlock: B,C,T,H,W = 2,128,8,64,64; E=8; top-2)
B, C, T, H, W = 2, 128, 8, 64, 64
E = 8
THW = T * H * W               # 32768
NVOX = B * THW                # 65536 voxels
NCORES = 8
NSH = NVOX // NCORES          # 8192 voxels per core
CPB = NCORES // B             # cores per batch element (4)
NSPLIT = 4                    # pipeline depth: device exec and both wire
                              # directions overlap across splits (the
                              # per-device queue is FIFO in issue order,
                              # but up- and down-transfers are full duplex)
NS = NSH // NSPLIT            # voxels per core per call
NC_CHUNK = 1024               # main-loop chunk (voxels)
NCHUNKS = NS // NC_CHUNK
F16 = mybir.dt.float16
F32 = mybir.dt.float32
F8 = mybir.dt.float8e4
NP_F8 = mybir.dt.np(F8)       # ml_dtypes.float8_e4m3

# ml_dtypes' elementwise f8 casts go through a slow scalar ufunc path on
# this 1-CPU host; table lookups are ~2x faster for both directions.
with np.errstate(invalid="ignore", over="ignore"):
    # f32 -> f16 (SIMD) -> f8 via 64K-entry table (double rounding is
    # harmless at our 13x error margin)
    _LUT_F16_TO_F8 = (np.arange(65536, dtype=np.uint16).view(np.float16)
                      .astype(NP_F8))
    # f8 -> f32 via 256-entry table
    _LUT_F8_TO_F32 = (np.arange(256, dtype=np.uint8).view(NP_F8)
                      .astype(np.float32))


def _to_f8(a32):
    """fast f32 ndarray -> f8e4m3 ndarray (any shape/strides)"""
    return _LUT_F16_TO_F8[a32.astype(np.float16).view(np.uint16)]


def _split_waits(nc, max_waits=1):
    """The walrus scheduler accepts only one sync-wait per instruction.
    Move extra on_wait conditions onto standalone same-engine NoOps
    inserted immediately before the instruction (same engine stream =>
    identical semantics)."""
    ctr = 0
    for f in nc.m.functions:
        for bb in f.blocks:
            insts = list(bb.instructions)
            out = []
            changed = False
            for inst in insts:
                si = inst.sync_info
                w = list(si.on_wait) if si is not None and si.on_wait else []
                if (len(w) > max_waits
                        and inst.engine != mybir.EngineType.Unassigned):
                    for extra in w[:-max_waits]:
                        ctr += 1
                        nop = mybir.InstNoOp(
                            name=f"WSPLIT-{ctr}", ins=[], outs=[])
                        nop.engine = inst.engine
                        nop.sync_info = mybir.SyncInfo(
                            on_wait=[extra], on_update=[])
                        out.append(nop)
                    inst.sync_info = mybir.SyncInfo(
                        on_wait=w[-max_waits:],
                        on_update=list(si.on_update) if si.on_update else [])
                    changed = True
                out.append(inst)
            if changed:
                try:
                    bb.instructions = out
                except Exception:
                    bb.instructions.clear()
                    bb.instructions.extend(out)
    return nc


def build_kernel(hasb1: bool, hasb2: bool):
    """Expert MLP only — gating (logits/top-2/softmax) happens on host in
    exact f32; the per-voxel expert weights arrive as the f8 `wc` input.
    Output is the MoE delta (no +x residual; host adds it in f32)."""
    act_fn = mybir.ActivationFunctionType.Silu
    wt_cols = 2 * E * C + (E if hasb1 else 0)
    nc = bass.Bass()
    x8_d = nc.dram_tensor("x8", [C, NS], F8, kind="ExternalInput")
    wc_d = nc.dram_tensor("wc", [E, NS], F8, kind="ExternalInput")
    wt_d = nc.dram_tensor("wt", [C, wt_cols], F16, kind="ExternalInput")
    b2_d = (nc.dram_tensor("b2m", [E, C], F16, kind="ExternalInput")
            if hasb2 else None)
    out_d = nc.dram_tensor("out", [C, NS], F8, kind="ExternalOutput")

    with tile.TileContext(nc) as tc:
        with (
            tc.tile_pool(name="consts", bufs=1) as consts,
            tc.tile_pool(name="xp", bufs=1) as xp,
            tc.tile_pool(name="fpool", bufs=3) as fpool,
            tc.tile_pool(name="gpool", bufs=3) as gpool,
            tc.tile_pool(name="opool", bufs=2) as opool,
        ):
            # ---------- loads ----------
            x_sb = xp.tile([C, NS], F8)
            wcm_sb = consts.tile([E, NS], F8)
            w1T = consts.tile([C, E * C], F16)
            w2T = consts.tile([C, E * C], F16)
            sel32 = consts.tile([E, E * C], F32)
            sel = consts.tile([E, E * C], F16)

            dmas = []
            for j in range(4):
                s = slice(j * (NS // 4), (j + 1) * (NS // 4))
                dmas.append(nc.sync.dma_start(x_sb[:, s], x8_d[:, s]))
            dmas.append(nc.sync.dma_start(wcm_sb[:], wc_d[:]))
            dmas.append(nc.sync.dma_start(w1T[:], wt_d[:, 0:E * C]))
            dmas.append(nc.sync.dma_start(w2T[:], wt_d[:, E * C:2 * E * C]))
            if hasb1:
                b1m = consts.tile([C, E], F32)
                bh = consts.tile([C, E], F16)
                dmas.append(nc.sync.dma_start(
                    bh[:], wt_d[:, 2 * E * C:2 * E * C + E]))
                nc.scalar.copy(b1m[:], bh[:])
            if hasb2:
                b2m = consts.tile([E, C], F16)
                dmas.append(nc.sync.dma_start(b2m[:], b2_d[:]))

            # on-device constant: expert-broadcast selector
            # sel[e, e*C:(e+1)*C] = 1, else 0
            nc.vector.memset(sel32[:], 1.0)
            nc.gpsimd.affine_select(
                sel32[:], sel32[:], pattern=[[1, E * C]], base=0,
                channel_multiplier=-C,
                compare_op=mybir.AluOpType.is_ge, fill=0.0)
            nc.gpsimd.affine_select(
                sel32[:], sel32[:], pattern=[[-1, E * C]], base=C - 1,
                channel_multiplier=C,
                compare_op=mybir.AluOpType.is_ge, fill=0.0)
            sel_cp = nc.scalar.copy(sel[:], sel32[:])

            # PE carries only ONE sync wait per Matmult through walrus;
            # absorb each input-DMA dependency into a PE nop up front.
            dma_nops = []
            for dma in dmas:
                nop = nc.tensor.nop(nofuse=True)
                add_dep_helper(nop.ins, dma.ins, sync=True)
                dma_nops.append(nop)

            def pe_absorb(producers, consumer_mms):
                nops = []
                for p in producers:
                    if p is None:
                        continue
                    n = nc.tensor.nop(nofuse=True)
                    add_dep_helper(n.ins, p.ins, sync=True)
                    nops.append(n)
                for m in consumer_mms:
                    for n in nops:
                        add_dep_helper(m.ins, n.ins, sync=False)

            # ---------- experts + combine ----------
            mpsum = tc.tile_pool(name="ps_m", bufs=1, space="PSUM")
            ps_m = mpsum.__enter__()
            mpsum2 = tc.tile_pool(name="ps_m2", bufs=2, space="PSUM")
            ps_m2 = mpsum2.__enter__()
            prev_out = prev_l2last = None
            hist_silu = [None, None]
            hist_mult = [None, None]
            hist_hmm = [None, None]
            hist_wb = [None, None]
            for i in range(NCHUNKS):
                cs = slice(i * NC_CHUNK, (i + 1) * NC_CHUNK)
                pso = ps_m.tile([C, NC_CHUNK], F32, tag="pso")
                for e in range(E):
                    mms = []
                    psh = ps_m2.tile([C, NC_CHUNK], F32, tag="psh")
                    for s in range(NC_CHUNK // 512):
                        rs = slice(i * NC_CHUNK + s * 512,
                                   i * NC_CHUNK + (s + 1) * 512)
                        mms.append(nc.tensor.matmul(
                            psh[:, s * 512:(s + 1) * 512],
                            w1T[:, e * C:(e + 1) * C],
                            x_sb[:, rs],
                            start=True, stop=True))
                    f_sb = fpool.tile([C, NC_CHUNK], F16, tag="f")
                    if hasb1:
                        silu_ins = nc.scalar.activation(
                            f_sb[:], psh[:], act_fn, bias=b1m[:, e:e + 1])
                    else:
                        silu_ins = nc.scalar.activation(f_sb[:], psh[:], act_fn)
                    pswb = ps_m.tile([C, NC_CHUNK], F32, tag="pswb")
                    for s in range(NC_CHUNK // 512):
                        ws = slice(i * NC_CHUNK + s * 512,
                                   i * NC_CHUNK + (s + 1) * 512)
                        mms.append(nc.tensor.matmul(
                            pswb[:, s * 512:(s + 1) * 512],
                            sel[:, e * C:(e + 1) * C], wcm_sb[:, ws],
                            start=True, stop=True))
                    g_sb = gpool.tile([C, NC_CHUNK], F16, tag="g")
                    mult_ins = nc.vector.tensor_mul(g_sb[:], f_sb[:], pswb[:])
                    for s in range(NC_CHUNK // 512):
                        ss = slice(s * 512, (s + 1) * 512)
                        mms.append(nc.tensor.matmul(
                            pso[:, ss],
                            w2T[:, e * C:(e + 1) * C],
                            g_sb[:, ss],
                            start=(e == 0),
                            stop=(e == E - 1) and not hasb2))
                    # absorb all cross-engine + psum-WAW deps into PE nops
                    pe_absorb([hist_silu[0], hist_mult[-1], prev_out,
                               hist_hmm[0], hist_wb[-1], prev_l2last,
                               sel_cp if (i == 0 and e == 0) else None],
                              mms[:1])
                    for m in mms[1:]:
                        add_dep_helper(m.ins, mms[0].ins, sync=False)
                    pe_absorb([silu_ins, mult_ins], mms[-NC_CHUNK // 512:])
                    hist_silu = [hist_silu[-1], silu_ins]
                    hist_mult = [hist_mult[-1], mult_ins]
                    hist_hmm = [hist_hmm[-1], mms[1]]
                    hist_wb = [hist_wb[-1], mms[NC_CHUNK // 512 + 1]]
                    if e == E - 1:
                        prev_l2last = mms[-1]
                if hasb2:
                    for s in range(NC_CHUNK // 512):
                        ss = slice(s * 512, (s + 1) * 512)
                        rs = slice(i * NC_CHUNK + s * 512,
                                   i * NC_CHUNK + (s + 1) * 512)
                        nc.tensor.matmul(
                            pso[:, ss], b2m[:], wcm_sb[:, rs],
                            start=False, stop=True)
                # delta only — the +x residual happens on host in f32
                o_sb = opool.tile([C, NC_CHUNK], F8, tag="o")
                prev_out = nc.scalar.copy(o_sb[:], pso[:])
                nc.sync.dma_start(out_d[:, cs], o_sb[:])
            mpsum2.__exit__(None, None, None)
            mpsum.__exit__(None, None, None)
    _split_waits(nc)
    return nc


# --- host side -------------------------------------------------------------
# Replicates bass_utils.run_bass_kernel_spmd's axon execution path
# (shard_map over jax.devices()[:8] + the _bass_exec_p custom call), but
# builds the jitted callable ONCE and reuses it: re-tracing and re-lowering
# on every call costs seconds of host time while computing nothing new.
_state: dict = {}


def _get_state(key):
    if key in _state:
        return _state[key]
    install_neuronx_cc_hook()
    nc = build_kernel(*key)
    partition_name = (nc.partition_id_tensor.name
                      if nc.partition_id_tensor else None)
    in_names, out_names, out_avals = [], [], []
    for alloc in nc.m.functions[0].allocations:
        if not isinstance(alloc, mybir.MemoryLocationSet):
            continue
        name = alloc.memorylocations[0].name
        if alloc.kind == "ExternalInput":
            if name != partition_name:
                in_names.append(name)
        elif alloc.kind == "ExternalOutput":
            out_names.append(name)
            out_avals.append(jax.core.ShapedArray(
                tuple(alloc.tensor_shape), mybir.dt.np(alloc.dtype)))
    n_params = len(in_names)
    n_outs = len(out_avals)
    all_names = tuple(in_names + out_names
                      + ([partition_name] if partition_name else []))

    def _body(*args):
        operands = list(args)
        if partition_name is not None:
            operands.append(partition_id_tensor())
        outs = _bass_exec_p.bind(
            *operands,
            out_avals=tuple(out_avals),
            in_names=all_names,
            out_names=tuple(out_names),
            lowering_input_output_aliases=(),
            sim_require_finite=True,
            sim_require_nnan=True,
            nc=nc,
        )
        return tuple(outs)

    devices = jax.devices()[:NCORES]
    mesh = Mesh(np.asarray(devices), ("core",))
    sharding = NamedSharding(mesh, PartitionSpec("core"))
    donate = tuple(range(n_params, n_params + n_outs))
    njit = jax.jit(
        shard_map(_body, mesh=mesh,
                  in_specs=(PartitionSpec("core"),) * (n_params + n_outs),
                  out_specs=(PartitionSpec("core"),) * n_outs,
                  check_rep=False),
        donate_argnums=donate, keep_unused=True)
    mkdonor = jax.jit(
        lambda: jnp.zeros((NCORES * C, NS), NP_F8), out_shardings=sharding)
    st = {"nc": nc, "njit": njit, "mkdonor": mkdonor, "sharding": sharding,
          "devices": devices, "in_names": in_names, "donors": [],
          "wt_sig": None, "wt_dev": None, "compiled": None}
    _state[key] = st
    return st


def kernel(x, gate_w, gate_b, w1, b1, w2, b2):
    x = np.asarray(x, dtype=np.float32)
    gate_w = np.asarray(gate_w, dtype=np.float32)
    gate_b = np.asarray(gate_b, dtype=np.float32)
    w1 = np.asarray(w1, dtype=np.float32)
    b1 = np.asarray(b1, dtype=np.float32)
    w2 = np.asarray(w2, dtype=np.float32)
    b2 = np.asarray(b2, dtype=np.float32)

    key = (bool(b1.any()), bool(b2.any()))
    st = _get_state(key)

    xr = x.reshape(B, C, THW)

    # x split shards, quantized to fp8, shipped as one async device_put
    # (a single put returns ~40ms sooner than 8 per-device puts on this
    # 1-CPU host, and the upload wire time is identical). The transfer is
    # in flight during the host gating compute below.
    x5 = x.reshape(B, C, CPB, NSPLIT, NS)

    def put_x(sp):
        # one strided gather+quantize pass: [B,CPB,C,NS] row-blocks land
        # exactly at core c = bb*CPB + q
        xg = _to_f8(x5[:, :, :, sp, :].transpose(0, 2, 1, 3)
                    ).reshape(NCORES * C, NS)
        return jax.device_put(xg, st["sharding"])

    x8_dev0 = put_x(0)   # upload in flight during the host work below

    # weights: device-resident across calls; re-uploaded only on change
    cached = st["wt_sig"]
    if (cached is None or not np.array_equal(cached[0], w1)
            or not np.array_equal(cached[1], w2)
            or not np.array_equal(cached[2], b1)):
        wt_cols = 2 * E * C + (E if key[0] else 0)
        wpk = np.empty((C, wt_cols), dtype=np.float16)
        wpk[:, 0:E * C] = w1.T
        wpk[:, E * C:2 * E * C] = w2.transpose(2, 0, 1).reshape(C, E * C)
        if key[0]:
            wpk[:, 2 * E * C:] = b1.reshape(E, C).T
        st["wt_dev"] = jax.device_put(np.tile(wpk, (NCORES, 1)),
                                      st["sharding"])
        st["wt_sig"] = (w1.copy(), w2.copy(), b1.copy())

    # --- gating on host, exact f32, one split at a time ---
    xr4 = x.reshape(B, C, CPB, NSH)
    has_gb = bool(gate_b.any())

    def gate_split(sp):
        lo = sp * NS
        # per-core strided-view gemms: 5x faster than tensordot (which
        # copies the slice before BLAS)
        lg = np.empty((E, NCORES, NS), dtype=np.float32)
        for c in range(NCORES):
            bb, q = divmod(c, CPB)
            o = q * NSH + lo
            np.matmul(gate_w, xr[bb, :, o:o + NS], out=lg[:, c, :])
        if has_gb:
            lg += gate_b[:, None, None]
        lt = lg.reshape(E, -1)
        n2 = lt.shape[1]
        ar = np.arange(n2)
        i1 = np.argmax(lt, axis=0)
        v1 = lt[i1, ar]
        lt[i1, ar] = -np.inf
        i2 = np.argmax(lt, axis=0)
        v2 = lt[i2, ar]
        ez = 1.0 / (1.0 + np.exp(v2 - v1))             # top-2 softmax
        w32 = np.zeros((E, n2), dtype=np.float32)
        w32[i1, ar] = ez
        w32[i2, ar] = 1.0 - ez
        return _to_f8(
            w32.reshape(E, NCORES, NS).transpose(1, 0, 2)
        ).reshape(NCORES * E, NS)

    b2t = (np.tile(b2.astype(np.float16), (NCORES, 1)) if key[1] else None)
    donors = st["donors"]
    st["donors"] = []

    def dispatch(sp, x8_dev):
        args_np = {"x8": x8_dev, "wc": gate_split(sp), "wt": st["wt_dev"],
                   "b2m": b2t}
        args = [args_np[n] for n in st["in_names"]]
        donor = donors.pop() if donors else st["mkdonor"]()
        fn = st["compiled"] or st["njit"]
        return fn(*args, donor)[0]

    # Issue order is the device queue order: dispatch split sp and kick
    # its result-fetch asyncs (pre-queued, so the data streams back the
    # moment exec sp finishes) BEFORE uploading split sp+1 — the fetch of
    # sp overlaps the upload of sp+1 on the full-duplex link.
    outs = [None] * NSPLIT
    all_shards = [None] * NSPLIT
    for sp in range(NSPLIT):
        outs[sp] = dispatch(sp, x8_dev0 if sp == 0 else put_x(sp))
        shards = sorted(outs[sp].addressable_shards,
                        key=lambda s: s.index[0].start or 0)
        all_shards[sp] = shards
        for s in shards:
            try:
                s.data.copy_to_host_async()
            except Exception:
                pass
    # interleave the f32 residual adds with the remaining transfers
    # (f8 -> f32 through the 256-entry table: ~2x faster than the
    # ml_dtypes mixed-dtype ufunc)
    out = np.empty((B, C, THW), dtype=np.float32)
    for sp in range(NSPLIT):
        for s in all_shards[sp]:
            c = (s.index[0].start or 0) // C
            bb, q = divmod(c, CPB)
            lo = q * NSH + sp * NS
            d32 = _LUT_F8_TO_F32[np.asarray(s.data).view(np.uint8)]
            np.add(xr[bb, :, lo:lo + NS], d32,
                   out=out[bb, :, lo:lo + NS])
        st["donors"].append(outs[sp])   # recycle as future donations
    return out.reshape(B, C, T, H, W)
